# revision 1
# baseline (speedup 1.0000x reference)
"""MoE routing kernel for 8 Trainium2 NeuronCores (Bass/Tile, SPMD).

Strategy (expert-parallel, matching the sharding hint):
  - Host computes the gate (softmax + top-2) and dispatches tokens: each of
    the 8 cores owns 2 of the 16 routed experts and receives only the tokens
    routed to its experts (gathered + transposed + zero-padded to a common
    capacity). This is the "all-to-all token dispatch on the topk indices".
  - The output layer (ow) is linear and commutes with the weighted combine,
    so it is folded into each expert's second matmul on the host
    (w2ot = w2[e].T @ ow.T), shrinking stage-2 work by W/OUT = 4x.
  - The shared expert is sharded over its intermediate dim (2048/8=256 rows
    per core); every core computes a partial for all 2048 tokens, also with
    ow folded in.  Bias terms that commute with the output layer
    (b2, sb2, ob) are applied analytically on the host.
  - Device matmuls run in float32r (full-rate fp32 on the PE array).
  - Host combines: scatter-add of combine-weight-scaled routed partials +
    shared partials + analytic bias terms.
"""
import sys

if "/opt/trn_rl_repo" not in sys.path:
    sys.path.insert(0, "/opt/trn_rl_repo")

import numpy as np
import concourse.bass as bass
import concourse.tile as tile
from concourse import mybir
from concourse.bass_utils import run_bass_kernel_spmd

B = 2048
W = 512
E = 16
TOPK = 2
INTER = 1024
SH = 2048
OUT = 128
NCORES = 8
EPC = E // NCORES          # experts per core = 2
SHS = SH // NCORES         # shared-expert inter slice per core = 256
KW = W // 128              # k-tiles over W = 4
MI = INTER // 128          # m-tiles over INTER = 8
MS = SHS // 128            # m-tiles over shared slice = 2
F32 = mybir.dt.float32
F32R = mybir.dt.float32r
F16 = mybir.dt.float16
DT = F16                   # device datapath dtype for matmul operands
NPDT = np.float16

# set by test.py to collect a profile; results stashed in LAST_RESULTS
TRACE = False
TRACE_KW = {}
LAST_RESULTS = None


def _legalize_waits(nc):
    """This container's walrus accepts at most 1 sync wait per instruction
    (2 for EventSemaphore).  Hoist excess waits emitted by the Tile
    scheduler into standalone EventSemaphore instructions."""
    for fn in nc.m.functions:
        for blk in fn.blocks:
            out = []
            changed = False
            for inst in blk.instructions:
                si = getattr(inst, "sync_info", None)
                waits = list(si.on_wait) if si is not None and si.on_wait else []
                cap = 2 if isinstance(inst, mybir.InstEventSemaphore) else 1
                if len(waits) > cap:
                    extra, keep = waits[:-cap], waits[-cap:]
                    for i in range(0, len(extra), 2):
                        out.append(mybir.InstEventSemaphore(
                            name=nc.get_next_instruction_name(),
                            engine=inst.engine,
                            ins=[], outs=[],
                            sync_info=mybir.SyncInfo(
                                on_wait=list(extra[i:i + 2]), on_update=[]),
                        ))
                    si.on_wait = keep
                    changed = True
                out.append(inst)
            if changed:
                blk.instructions = out


def _token_chunks(cap):
    """Split [0, cap) into chunks of <=512 (all multiples of 128)."""
    chunks = []
    off = 0
    while off < cap:
        sz = min(512, cap - off)
        chunks.append((off, sz))
        off += sz
    return chunks


def _build_nc(cap, loop_n=None, legalize=True, mode="full"):
    """Build the SPMD Bass program for per-expert token capacity `cap`
    (multiple of 128).  loop_n wraps the body in a hardware For_i loop
    (used only for timing measurements)."""
    nc = bass.Bass("TRN2", target_bir_lowering=False, debug=False)

    def din(name, f, dt=DT):
        return nc.dram_tensor(name, [128, f], dt, kind="ExternalInput").ap()

    xt = din("xt", KW * B)                   # x.T packed: col block k = x.T[128k:128k+128, :]
    xg = din("xg", EPC * KW * cap)           # gathered tokens per expert, packed like xt
    w1t = din("w1t", EPC * KW * INTER)       # per expert: w1[e].T packed k-blocks
    w3t = din("w3t", EPC * KW * INTER)
    w2ot = din("w2ot", EPC * MI * OUT)       # per expert: (w2[e].T @ ow.T) packed k-blocks
    sw1t = din("sw1t", KW * SHS)             # shared slice: sw1[s].T packed
    sw3t = din("sw3t", KW * SHS)
    sw2ot = din("sw2ot", MS * OUT)           # (sw2[:, s].T @ ow.T) packed
    bias = din("bias", EPC * 2 * MI + 2 * MS, F32)  # b1/b3 per expert (8 cols each), sb1/sb3 (2 cols each)

    yr = nc.dram_tensor("yr", [128, EPC * cap], F32, kind="ExternalOutput").ap()
    zt = nc.dram_tensor("zt", [128, B], F32, kind="ExternalOutput").ap()

    LR = mybir.ActivationFunctionType.Lrelu
    IDT = mybir.ActivationFunctionType.Identity

    with tile.TileContext(nc) as tc:
        import contextlib
        with tc.tile_pool(name="wts", bufs=1) as wts, \
             tc.tile_pool(name="work", bufs=2) as work, \
             tc.tile_pool(name="hts", bufs=1) as hts, \
             tc.tile_pool(name="outs", bufs=2) as outs, \
             tc.tile_pool(name="ps", bufs=2, space="PSUM") as ps, \
             contextlib.ExitStack() as _loopstack:
            _loop_entered = [False]

            def _enter_loop():
                if loop_n is not None and not _loop_entered[0]:
                    _loopstack.enter_context(tc.For_i(
                        0, loop_n, 1,
                        hint_engines=(mybir.EngineType.PE,
                                      mybir.EngineType.Activation,
                                      mybir.EngineType.DVE,
                                      mybir.EngineType.SP)))
                    _loop_entered[0] = True
            if mode != "compute":
                _enter_loop()

            def emit_body():
              # ---- tiny bias + shared-expert inputs go on otherwise-idle DMA
              # queues (Pool/ACT) so the SP queue is dedicated to expert weights.
              bias_t = wts.tile([128, bias.shape[1]], F32, tag="bias")
              nc.scalar.dma_start(bias_t[:], bias[:])
              sw1_ts, sw3_ts = [], []
              for k in range(KW):
                  t = wts.tile([128, SHS], DT, tag=f"sw1k{k}")
                  nc.scalar.dma_start(t[:], sw1t[:, k * SHS:(k + 1) * SHS])
                  sw1_ts.append(t)
                  t = wts.tile([128, SHS], DT, tag=f"sw3k{k}")
                  nc.scalar.dma_start(t[:], sw3t[:, k * SHS:(k + 1) * SHS])
                  sw3_ts.append(t)
              sw2_t = wts.tile([128, MS * OUT], DT, tag="sw2")
              nc.scalar.dma_start(sw2_t[:], sw2ot[:])
              xt_ts = []
              for k in range(KW):
                  t = wts.tile([128, B], DT, tag=f"xtk{k}")
                  nc.scalar.dma_start(t[:], xt[:, k * B:(k + 1) * B])
                  xt_ts.append(t)

              def b_ap(col):  # [128,1] per-partition bias column
                  return bias_t[:, col:col + 1]

              chunks = _token_chunks(cap)

              def expert_steps(e):
                  """Generator: step 0 = weight DMAs, then one step per
                  (token-chunk, m-tile) with stage-2 interleaved."""
                  w1_ts, w3_ts, xg_ts = [], [], []
                  for k in range(KW):
                      t = work.tile([128, INTER], DT, tag=f"w1k{k}", bufs=3)
                      nc.sync.dma_start(t[:], w1t[:, (e * KW + k) * INTER:(e * KW + k + 1) * INTER])
                      w1_ts.append(t)
                      t = work.tile([128, INTER], DT, tag=f"w3k{k}", bufs=3)
                      nc.sync.dma_start(t[:], w3t[:, (e * KW + k) * INTER:(e * KW + k + 1) * INTER])
                      w3_ts.append(t)
                      t = work.tile([128, cap], DT, tag=f"xgk{k}")
                      nc.sync.dma_start(t[:], xg[:, (e * KW + k) * cap:(e * KW + k + 1) * cap])
                      xg_ts.append(t)
                  w2_t = work.tile([128, MI * OUT], DT, tag="w2")
                  nc.sync.dma_start(w2_t[:], w2ot[:, e * MI * OUT:(e + 1) * MI * OUT])
                  yield

                  LAG = 2   # stage-2 MMs trail stage 1 so the in-order PE
                            # stream never stalls on the ACT->DVE h chain
                  for (c0, csz) in chunks:
                      py = ps.tile([128, csz], F32, tag="py")
                      hts_pend = []
                      for m in range(MI):
                          p1 = ps.tile([128, csz], F32, tag="p1", bufs=3)
                          p3 = ps.tile([128, csz], F32, tag="p3", bufs=3)
                          for k in range(KW):
                              lhs1 = w1_ts[k][:, m * 128:(m + 1) * 128]
                              rhs = xg_ts[k][:, c0:c0 + csz]
                              nc.tensor.matmul(p1[:], lhs1, rhs, start=(k == 0), stop=(k == KW - 1))
                          for k in range(KW):
                              lhs3 = w3_ts[k][:, m * 128:(m + 1) * 128]
                              rhs = xg_ts[k][:, c0:c0 + csz]
                              nc.tensor.matmul(p3[:], lhs3, rhs, start=(k == 0), stop=(k == KW - 1))
                          a = work.tile([128, csz], DT, tag="act_a")
                          nc.scalar.activation(a[:], p1[:], LR, bias=b_ap(e * 2 * MI + m), alpha=0.01)
                          t3 = work.tile([128, csz], DT, tag="act_b")
                          nc.vector.tensor_scalar_add(t3[:], p3[:], b_ap(e * 2 * MI + MI + m))
                          ht = hts.tile([128, csz], DT, tag=f"ht{m}", bufs=3)
                          nc.vector.tensor_mul(ht[:], a[:], t3[:])
                          hts_pend.append((m, ht))
                          if len(hts_pend) > LAG:
                              md, htd = hts_pend.pop(0)
                              lhs = w2_t[:, md * OUT:(md + 1) * OUT]
                              nc.tensor.matmul(py[:], lhs, htd[:], start=(md == 0), stop=(md == MI - 1))
                          if not (m == MI - 1 and (c0, csz) == chunks[-1]):
                              yield
                      for md, htd in hts_pend:
                          lhs = w2_t[:, md * OUT:(md + 1) * OUT]
                          nc.tensor.matmul(py[:], lhs, htd[:], start=(md == 0), stop=(md == MI - 1))
                      yo = outs.tile([128, csz], F32, tag="yo")
                      nc.vector.tensor_copy(yo[:], py[:])
                      nc.sync.dma_start(yr[:, e * cap + c0: e * cap + c0 + csz], yo[:])
                  yield

              def shared_steps():
                  """Generator: one step per (token-group, m-tile); the pz
                  stage-2 accumulation trails by one group to avoid PE stalls."""
                  pend = []   # (pz, zo-flush closure) per group

                  def flush(pzg, gc0, gcsz, hs_list):
                      for m, hs in hs_list:
                          lhs = sw2_t[:, m * OUT:(m + 1) * OUT]
                          nc.tensor.matmul(pzg[:], lhs, hs[:], start=(m == 0), stop=(m == MS - 1))
                      zo = outs.tile([128, gcsz], F32, tag="zo")
                      nc.vector.tensor_copy(zo[:], pzg[:])
                      nc.sync.dma_start(zt[:, gc0:gc0 + gcsz], zo[:])

                  for (c0, csz) in _token_chunks(B):
                      pz = ps.tile([128, csz], F32, tag="py")
                      hs_list = []
                      for m in range(MS):
                          p1 = ps.tile([128, csz], F32, tag="p1", bufs=3)
                          p3 = ps.tile([128, csz], F32, tag="p3", bufs=3)
                          for k in range(KW):
                              lhs1 = sw1_ts[k][:, m * 128:(m + 1) * 128]
                              rhs = xt_ts[k][:, c0:c0 + csz]
                              nc.tensor.matmul(p1[:], lhs1, rhs, start=(k == 0), stop=(k == KW - 1))
                          for k in range(KW):
                              lhs3 = sw3_ts[k][:, m * 128:(m + 1) * 128]
                              rhs = xt_ts[k][:, c0:c0 + csz]
                              nc.tensor.matmul(p3[:], lhs3, rhs, start=(k == 0), stop=(k == KW - 1))
                          a = work.tile([128, csz], DT, tag="act_a")
                          nc.scalar.activation(a[:], p1[:], LR, bias=b_ap(EPC * 2 * MI + m), alpha=0.01)
                          t3 = work.tile([128, csz], DT, tag="act_b")
                          nc.vector.tensor_scalar_add(t3[:], p3[:], b_ap(EPC * 2 * MI + MS + m))
                          hs = hts.tile([128, csz], DT, tag=f"hs{m}", bufs=3)
                          nc.vector.tensor_mul(hs[:], a[:], t3[:])
                          hs_list.append((m, hs))
                          yield
                      pend.append((pz, c0, csz, hs_list))
                      if len(pend) > 1:
                          flush(*pend.pop(0))
                  for args_ in pend:
                      flush(*args_)
                  yield

              # fine-grained interleave of the expert stream (SP DMA queue) and
              # the shared-expert stream (ACT DMA queue): emission order sets
              # scheduler priority, so merging at m-tile granularity lets each
              # stream fill PE stalls caused by the other's weight DMAs.
              import os as _os
              if mode == "dma":
                  for g in [expert_steps(e) for e in range(EPC)]:
                      next(g)   # DMA prologue only
              elif mode == "experts":
                  for g in [expert_steps(e) for e in range(EPC)]:
                      for _ in g:
                          pass
              elif mode == "shared":
                  for _ in shared_steps():
                      pass
              else:
                  pattern = _os.environ.get(
                      "K_PATTERN", "E ESSE EES EES EE ESSE EES EES EE SSSS")
                  elist = [expert_steps(e) for e in range(EPC)]
                  if mode == "compute":
                      for g in elist:
                          next(g)          # emit weight DMAs outside the loop
                      _enter_loop()        # loop wraps compute only
                  sgen = shared_steps()
                  ei = 0
                  for ch in pattern:
                      if ch == " ":
                          continue
                      if ch == "E":
                          if elist[ei] is None:
                              continue
                          try:
                              next(elist[ei])
                          except StopIteration:
                              elist[ei] = None
                              ei = min(ei + 1, EPC - 1)
                      else:
                          try:
                              next(sgen)
                          except StopIteration:
                              pass
                  for g in elist + [sgen]:
                      if g is None:
                          continue
                      for _ in g:
                          pass


            if mode.startswith("u"):
                for _r in range(int(mode[1:])):
                    emit_body()
            else:
                emit_body()

    if legalize:
        _legalize_waits(nc)
    return nc


_NC_CACHE = {}


def _pack_kblocks(mat):
    """[Ktot, F] -> [128, (Ktot/128)*F] with col block k = mat[128k:128(k+1), :]."""
    ktot, f = mat.shape
    assert ktot % 128 == 0
    return np.ascontiguousarray(
        mat.reshape(ktot // 128, 128, f).transpose(1, 0, 2).reshape(128, -1))


def prepare(x, task_id, gate_w, w1, b1, w2, b2, w3, b3,
            sw1, sb1, sw2, sb2, sw3, sb3, ow, ob):
    """Host-side routing + packing.  Returns everything needed to launch the
    device program and combine its partial outputs."""
    x = np.asarray(x, np.float32)
    f32 = lambda a: np.asarray(a, np.float32)
    gate_w, w1, b1, w2, b2, w3, b3 = map(f32, (gate_w, w1, b1, w2, b2, w3, b3))
    sw1, sb1, sw2, sb2, sw3, sb3, ow, ob = map(f32, (sw1, sb1, sw2, sb2, sw3, sb3, ow, ob))

    # ---- host gate: softmax + top-2 (the routing decision) ----
    logits = x @ gate_w.T
    logits -= logits.max(axis=1, keepdims=True)
    ex = np.exp(logits)
    scores = ex / ex.sum(axis=1, keepdims=True)            # [B, E] fp32
    order = np.argsort(-scores, axis=1, kind="stable")[:, :TOPK]   # [B, 2]

    tok_lists = []
    for e in range(E):
        sel = np.nonzero((order == e).any(axis=1))[0]
        tok_lists.append(sel)
    max_cnt = max(len(t) for t in tok_lists)
    cap = max(128, -(-max_cnt // 16) * 16)

    if cap not in _NC_CACHE:
        _NC_CACHE[cap] = _build_nc(cap)
    nc = _NC_CACHE[cap]

    # ---- pack per-core inputs (device datapath dtype) ----
    xt_p = _pack_kblocks(x.T.copy()).astype(NPDT)          # [128, KW*B]
    in_maps = []
    for c in range(NCORES):
        exps = [c * EPC + j for j in range(EPC)]
        xg_blocks, w1_bl, w3_bl, w2_bl = [], [], [], []
        bias_cols = []
        for e in exps:
            toks = tok_lists[e]
            xge = np.zeros((W, cap), np.float32)
            xge[:, :len(toks)] = x[toks].T
            xg_blocks.append(_pack_kblocks(xge).astype(NPDT))
            w1_bl.append(_pack_kblocks(w1[e].T.copy()).astype(NPDT))
            w3_bl.append(_pack_kblocks(w3[e].T.copy()).astype(NPDT))
            w2_bl.append(_pack_kblocks(w2[e].T @ ow.T).astype(NPDT))
        for e in exps:
            bias_cols.append(b1[e].reshape(MI, 128).T)     # [128, MI]
            bias_cols.append(b3[e].reshape(MI, 128).T)
        s = slice(c * SHS, (c + 1) * SHS)
        bias_cols.append(sb1[s].reshape(MS, 128).T)
        bias_cols.append(sb3[s].reshape(MS, 128).T)
        in_maps.append({
            "xt": xt_p,
            "xg": np.concatenate(xg_blocks, axis=1),
            "w1t": np.concatenate(w1_bl, axis=1),
            "w3t": np.concatenate(w3_bl, axis=1),
            "w2ot": np.concatenate(w2_bl, axis=1),
            "sw1t": _pack_kblocks(sw1[s].T.copy()).astype(NPDT),
            "sw3t": _pack_kblocks(sw3[s].T.copy()).astype(NPDT),
            "sw2ot": _pack_kblocks(sw2[:, s].T @ ow.T).astype(NPDT),
            "bias": np.ascontiguousarray(np.concatenate(bias_cols, axis=1)),
        })

    # dense combine weights [B, E] (zero except the top-2 experts per token)
    combine_w = np.zeros((B, E), np.float32)
    rows = np.arange(B)
    combine_w[rows[:, None], order] = np.take_along_axis(scores, order, axis=1)
    # analytic bias terms: sum_e combine[:,e] * (b2[e] @ ow.T)  +  sb2 @ ow.T + ob
    base = combine_w @ (b2 @ ow.T) + sb2 @ ow.T + ob

    return dict(nc=nc, cap=cap, in_maps=in_maps, tok_lists=tok_lists,
                combine_w=combine_w, base=base)


def combine(p, results):
    """Combine per-core device partials into the full [B, OUT] output."""
    cap, tok_lists, combine_w = p["cap"], p["tok_lists"], p["combine_w"]
    out = p["base"].astype(np.float32).copy()
    for c in range(NCORES):
        r = results[c]
        out += r["zt"].astype(np.float32).T
        for j in range(EPC):
            e = c * EPC + j
            toks = tok_lists[e]
            yre = r["yr"][:, j * cap: j * cap + len(toks)].astype(np.float32)  # [OUT, cnt]
            out[toks] += combine_w[toks, e][:, None] * yre.T
    return out


def kernel(x, task_id, gate_w, w1, b1, w2, b2, w3, b3,
           sw1, sb1, sw2, sb2, sw3, sb3, ow, ob):
    global LAST_RESULTS
    p = prepare(x, task_id, gate_w, w1, b1, w2, b2, w3, b3,
                sw1, sb1, sw2, sb2, sw3, sb3, ow, ob)
    res = run_bass_kernel_spmd(
        p["nc"], p["in_maps"], core_ids=list(range(NCORES)),
        trace=TRACE, **TRACE_KW)
    LAST_RESULTS = res
    return combine(p, res.results)



# revision 6
# speedup vs baseline: 1.2018x; 1.2018x over previous
"""MoE routing kernel for 8 Trainium2 NeuronCores (Bass/Tile, SPMD).

Strategy (expert-parallel, matching the sharding hint):
  - Host computes the gate (softmax + top-2) and dispatches tokens: each of
    the 8 cores owns 2 of the 16 routed experts and receives only the tokens
    routed to its experts (gathered + transposed + zero-padded to a slot
    capacity).  Experts are paired hot+cold across cores so the two slot
    capacities (cap1 >= cap2) are as small as possible.
  - The output layer (ow) is linear and commutes with the weighted combine,
    so it is folded into each expert's second matmul on the host
    (w2ot = w2[e].T @ ow.T), shrinking stage-2 work by W/OUT = 4x.
  - The shared expert is sharded over its intermediate dim (2048/8=256 rows
    per core); every core computes a partial for all 2048 tokens, also with
    ow folded in.  Bias terms that commute with the output layer
    (b2, sb2, ob) are applied analytically on the host.
  - Schedule: the shared expert runs FIRST (its inputs are small, so the PE
    starts almost immediately) while the bulky routed-expert weights stream
    in behind it; the routed experts then run back-to-back.  All DMAs are
    emitted in first-use order across multiple queues.
  - Host combines: scatter-add of combine-weight-scaled routed partials +
    shared partials + analytic bias terms.
"""
import sys

if "/opt/trn_rl_repo" not in sys.path:
    sys.path.insert(0, "/opt/trn_rl_repo")

import numpy as np
import concourse.bass as bass
import concourse.tile as tile
from concourse import mybir
from concourse.bass_utils import run_bass_kernel_spmd

B = 2048
W = 512
E = 16
TOPK = 2
INTER = 1024
SH = 2048
OUT = 128
NCORES = 8
EPC = E // NCORES          # expert slots per core = 2
SHS = SH // NCORES         # shared-expert inter slice per core = 256
KW = W // 128              # k-tiles over W = 4
MI = INTER // 128          # m-tiles over INTER = 8
MS = SHS // 128            # m-tiles over shared slice = 2
CH = 512                   # token chunk for the shared expert (PSUM bank)
NCH = B // CH
F32 = mybir.dt.float32
F16 = mybir.dt.float16
DT = F16                   # device datapath dtype for matmul operands
NPDT = np.float16

# set by test.py to collect a profile; results stashed in LAST_RESULTS
TRACE = False
TRACE_KW = {}
LAST_RESULTS = None


def _legalize_waits(nc):
    """This container's walrus accepts at most 1 sync wait per instruction
    (2 for EventSemaphore).  Hoist excess waits emitted by the Tile
    scheduler into standalone EventSemaphore instructions."""
    for fn in nc.m.functions:
        for blk in fn.blocks:
            out = []
            changed = False
            for inst in blk.instructions:
                si = getattr(inst, "sync_info", None)
                waits = list(si.on_wait) if si is not None and si.on_wait else []
                cap = 2 if isinstance(inst, mybir.InstEventSemaphore) else 1
                if len(waits) > cap:
                    extra, keep = waits[:-cap], waits[-cap:]
                    for i in range(0, len(extra), 2):
                        out.append(mybir.InstEventSemaphore(
                            name=nc.get_next_instruction_name(),
                            engine=inst.engine,
                            ins=[], outs=[],
                            sync_info=mybir.SyncInfo(
                                on_wait=list(extra[i:i + 2]), on_update=[]),
                        ))
                    si.on_wait = keep
                    changed = True
                out.append(inst)
            if changed:
                blk.instructions = out


def _build_nc(caps, legalize=True):
    """Build the SPMD Bass program for per-slot token capacities
    `caps = (cap1, cap2)` (multiples of 16, each <= 512)."""
    nc = bass.Bass("TRN2", target_bir_lowering=False, debug=False)
    capsum = sum(caps)

    def din(name, f, dt=DT):
        return nc.dram_tensor(name, [128, f], dt, kind="ExternalInput").ap()

    # shared expert + all tokens (needed first)
    bias = din("bias", EPC * 2 * MI + 2 * MS, F32)  # b1/b3 per slot, sb1/sb3
    sw1t = din("sw1t", KW * SHS)             # shared slice: sw1[s].T packed k-blocks
    sw3t = din("sw3t", KW * SHS)
    sw2ot = din("sw2ot", MS * OUT)           # (sw2[:, s].T @ ow.T) packed
    xtc = din("xtc", B * KW)                 # x.T packed per chunk: chunk c at
    #   cols [c*KW*CH, (c+1)*KW*CH), k-block k at [c*KW*CH + k*CH, ... + CH)
    # routed expert slots
    w1s = [din(f"w1t{j}", KW * INTER) for j in range(EPC)]
    w3s = [din(f"w3t{j}", KW * INTER) for j in range(EPC)]
    xgs = [din(f"xg{j}", KW * caps[j]) for j in range(EPC)]
    w2s = [din(f"w2ot{j}", MI * OUT) for j in range(EPC)]

    yr = nc.dram_tensor("yr", [128, capsum], F16, kind="ExternalOutput").ap()
    zt = nc.dram_tensor("zt", [128, B], F16, kind="ExternalOutput").ap()

    LR = mybir.ActivationFunctionType.Lrelu

    with tile.TileContext(nc) as tc:
        with tc.tile_pool(name="wts", bufs=1) as wts, \
             tc.tile_pool(name="work", bufs=3) as work, \
             tc.tile_pool(name="hts", bufs=1) as hts, \
             tc.tile_pool(name="outs", bufs=2) as outs, \
             tc.tile_pool(name="ps", bufs=2, space="PSUM") as ps:

            # ---- activation-table preload: a tiny LRELU on a memset tile
            # so the mid-kernel first LEAKY_RELU doesn't pay ACT_TABLE_LOAD.
            warm = wts.tile([128, 1], F32, tag="warm")
            nc.gpsimd.memset(warm[:], 0.0)
            warm2 = wts.tile([128, 1], DT, tag="warm2")
            nc.scalar.activation(warm2[:], warm[:], LR, alpha=0.01)

            # ---- DMAs in first-use order, spread across queues ----
            bias_t = wts.tile([128, bias.shape[1]], F32, tag="bias")
            nc.gpsimd.dma_start(bias_t[:], bias[:])
            sw1_t = wts.tile([128, KW * SHS], DT, tag="sw1")
            nc.scalar.dma_start(sw1_t[:], sw1t[:])
            xt_ts = []
            for c in range(NCH):
                t = wts.tile([128, KW * CH], DT, tag=f"xtc{c}")
                nc.sync.dma_start(t[:], xtc[:, c * KW * CH:(c + 1) * KW * CH])
                xt_ts.append(t)
            sw3_t = wts.tile([128, KW * SHS], DT, tag="sw3")
            nc.scalar.dma_start(sw3_t[:], sw3t[:])
            sw2_t = wts.tile([128, MS * OUT], DT, tag="sw2")
            nc.scalar.dma_start(sw2_t[:], sw2ot[:])
            # expert slots stream behind the shared phase
            w1_ts, w3_ts, xg_ts, w2_ts = [], [], [], []
            for j in range(EPC):
                eng = nc.scalar if j == 0 else nc.sync
                t = wts.tile([128, KW * INTER], DT, tag=f"w1s{j}")
                eng.dma_start(t[:], w1s[j][:])
                w1_ts.append(t)
                t = wts.tile([128, KW * caps[j]], DT, tag=f"xgs{j}")
                nc.gpsimd.dma_start(t[:], xgs[j][:])
                xg_ts.append(t)
                t = wts.tile([128, KW * INTER], DT, tag=f"w3s{j}")
                eng.dma_start(t[:], w3s[j][:])
                w3_ts.append(t)
                t = wts.tile([128, MI * OUT], DT, tag=f"w2s{j}")
                nc.gpsimd.dma_start(t[:], w2s[j][:])
                w2_ts.append(t)

            def b_ap(col):  # [128,1] per-partition bias column
                return bias_t[:, col:col + 1]

            def epilogue(p1, p3, bcol1, bcol3, csz, htag):
                """h = lrelu(p1 + b1) * (p3 + b3), split over 3 engines."""
                a = work.tile([128, csz], DT, tag="act_a")
                nc.scalar.activation(a[:], p1[:], LR, bias=b_ap(bcol1), alpha=0.01)
                t3 = work.tile([128, csz], DT, tag="act_b")
                nc.vector.tensor_scalar_add(t3[:], p3[:], b_ap(bcol3))
                ht = hts.tile([128, csz], DT, tag=htag, bufs=3)
                nc.vector.tensor_mul(ht[:], a[:], t3[:])
                return ht

            # ---- shared expert: 4 chunks of 512 tokens, stage-2 trails by
            # one chunk so the in-order PE stream never waits on the DVE.
            pend = None

            def flush_shared(pz, c, hs_list):
                for m, hs in hs_list:
                    nc.tensor.matmul(pz[:], sw2_t[:, m * OUT:(m + 1) * OUT],
                                     hs[:], start=(m == 0), stop=(m == MS - 1))
                zo = outs.tile([128, CH], F16, tag="zo")
                nc.vector.tensor_copy(zo[:], pz[:])
                nc.sync.dma_start(zt[:, c * CH:(c + 1) * CH], zo[:])

            for c in range(NCH):
                pz = ps.tile([128, CH], F32, tag="py")
                hs_list = []
                for m in range(MS):
                    p1 = ps.tile([128, CH], F32, tag="p1", bufs=3)
                    p3 = ps.tile([128, CH], F32, tag="p3", bufs=3)
                    for k in range(KW):
                        nc.tensor.matmul(p1[:], sw1_t[:, (k * MS + m) * 128:(k * MS + m + 1) * 128],
                                         xt_ts[c][:, k * CH:(k + 1) * CH],
                                         start=(k == 0), stop=(k == KW - 1))
                    for k in range(KW):
                        nc.tensor.matmul(p3[:], sw3_t[:, (k * MS + m) * 128:(k * MS + m + 1) * 128],
                                         xt_ts[c][:, k * CH:(k + 1) * CH],
                                         start=(k == 0), stop=(k == KW - 1))
                    hs = epilogue(p1, p3, EPC * 2 * MI + m, EPC * 2 * MI + MS + m,
                                  CH, f"hs{m}")
                    hs_list.append((m, hs))
                if pend is not None:
                    flush_shared(*pend)
                pend = (pz, c, hs_list)
            flush_shared(*pend)

            # ---- routed experts: stage-2 trails stage-1 by LAG m-tiles.
            LAG = 2
            for j in range(EPC):
                cap = caps[j]
                py = ps.tile([128, cap], F32, tag="py")
                hts_pend = []
                for m in range(MI):
                    p1 = ps.tile([128, cap], F32, tag="p1", bufs=3)
                    p3 = ps.tile([128, cap], F32, tag="p3", bufs=3)
                    for k in range(KW):
                        nc.tensor.matmul(p1[:], w1_ts[j][:, (k * MI + m) * 128:(k * MI + m + 1) * 128],
                                         xg_ts[j][:, k * cap:(k + 1) * cap],
                                         start=(k == 0), stop=(k == KW - 1))
                    for k in range(KW):
                        nc.tensor.matmul(p3[:], w3_ts[j][:, (k * MI + m) * 128:(k * MI + m + 1) * 128],
                                         xg_ts[j][:, k * cap:(k + 1) * cap],
                                         start=(k == 0), stop=(k == KW - 1))
                    ht = epilogue(p1, p3, j * 2 * MI + m, j * 2 * MI + MI + m,
                                  cap, f"ht{m}")
                    hts_pend.append((m, ht))
                    if len(hts_pend) > LAG:
                        md, htd = hts_pend.pop(0)
                        nc.tensor.matmul(py[:], w2_ts[j][:, md * OUT:(md + 1) * OUT],
                                         htd[:], start=(md == 0), stop=(md == MI - 1))
                for md, htd in hts_pend:
                    nc.tensor.matmul(py[:], w2_ts[j][:, md * OUT:(md + 1) * OUT],
                                     htd[:], start=(md == 0), stop=(md == MI - 1))
                yo = outs.tile([128, cap], F16, tag="yo")
                nc.vector.tensor_copy(yo[:], py[:])
                off = sum(caps[:j])
                nc.sync.dma_start(yr[:, off:off + cap], yo[:])

    if legalize:
        _legalize_waits(nc)
    return nc


_NC_CACHE = {}


def _pack_kblocks(mat):
    """[Ktot, F] -> [128, (Ktot/128)*F] with col block k = mat[128k:128(k+1), :]."""
    ktot, f = mat.shape
    assert ktot % 128 == 0
    return np.ascontiguousarray(
        mat.reshape(ktot // 128, 128, f).transpose(1, 0, 2).reshape(128, -1))


def _ceil16(n):
    return max(128, -(-n // 16) * 16)


def prepare(x, task_id, gate_w, w1, b1, w2, b2, w3, b3,
            sw1, sb1, sw2, sb2, sw3, sb3, ow, ob):
    """Host-side routing + packing.  Returns everything needed to launch the
    device program and combine its partial outputs."""
    x = np.asarray(x, np.float32)
    f32 = lambda a: np.asarray(a, np.float32)
    gate_w, w1, b1, w2, b2, w3, b3 = map(f32, (gate_w, w1, b1, w2, b2, w3, b3))
    sw1, sb1, sw2, sb2, sw3, sb3, ow, ob = map(f32, (sw1, sb1, sw2, sb2, sw3, sb3, ow, ob))

    # ---- host gate: softmax + top-2 (the routing decision) ----
    logits = x @ gate_w.T
    logits -= logits.max(axis=1, keepdims=True)
    ex = np.exp(logits)
    scores = ex / ex.sum(axis=1, keepdims=True)            # [B, E] fp32
    order = np.argsort(-scores, axis=1, kind="stable")[:, :TOPK]   # [B, 2]

    tok_lists = []
    for e in range(E):
        sel = np.nonzero((order == e).any(axis=1))[0]
        tok_lists.append(sel)

    # hot+cold pairing: core i gets (rank i, rank 15-i) by token count
    rank = sorted(range(E), key=lambda e: -len(tok_lists[e]))
    slot_exp = [[rank[i], rank[E - 1 - i]] for i in range(NCORES)]
    caps = tuple(_ceil16(max(len(tok_lists[slot_exp[c][j]]) for c in range(NCORES)))
                 for j in range(EPC))

    if caps not in _NC_CACHE:
        _NC_CACHE[caps] = _build_nc(caps)
    nc = _NC_CACHE[caps]

    # ---- pack per-core inputs (device datapath dtype) ----
    # xtc: chunk-major, then k-block: [128, NCH * KW * CH]
    xt_k = x.T.reshape(KW, 128, NCH, CH)                   # [k, p, c, t]
    xtc = np.ascontiguousarray(xt_k.transpose(1, 2, 0, 3).reshape(128, -1)).astype(NPDT)
    in_maps = []
    for c in range(NCORES):
        m = {"xtc": xtc}
        bias_cols = []
        for j in range(EPC):
            e = slot_exp[c][j]
            cap = caps[j]
            toks = tok_lists[e]
            xge = np.zeros((W, cap), np.float32)
            xge[:, :len(toks)] = x[toks].T
            m[f"xg{j}"] = _pack_kblocks(xge).astype(NPDT)
            m[f"w1t{j}"] = _pack_kblocks(w1[e].T.copy()).astype(NPDT)
            m[f"w3t{j}"] = _pack_kblocks(w3[e].T.copy()).astype(NPDT)
            m[f"w2ot{j}"] = _pack_kblocks(w2[e].T @ ow.T).astype(NPDT)
        for j in range(EPC):
            e = slot_exp[c][j]
            bias_cols.append(b1[e].reshape(MI, 128).T)     # [128, MI]
            bias_cols.append(b3[e].reshape(MI, 128).T)
        s = slice(c * SHS, (c + 1) * SHS)
        bias_cols.append(sb1[s].reshape(MS, 128).T)
        bias_cols.append(sb3[s].reshape(MS, 128).T)
        m["bias"] = np.ascontiguousarray(np.concatenate(bias_cols, axis=1))
        m["sw1t"] = _pack_kblocks(sw1[s].T.copy()).astype(NPDT)
        m["sw3t"] = _pack_kblocks(sw3[s].T.copy()).astype(NPDT)
        m["sw2ot"] = _pack_kblocks(sw2[:, s].T @ ow.T).astype(NPDT)
        in_maps.append(m)

    # dense combine weights [B, E] (zero except the top-2 experts per token)
    combine_w = np.zeros((B, E), np.float32)
    rows = np.arange(B)
    combine_w[rows[:, None], order] = np.take_along_axis(scores, order, axis=1)
    # analytic bias terms: sum_e combine[:,e] * (b2[e] @ ow.T)  +  sb2 @ ow.T + ob
    base = combine_w @ (b2 @ ow.T) + sb2 @ ow.T + ob

    return dict(nc=nc, caps=caps, slot_exp=slot_exp, in_maps=in_maps,
                tok_lists=tok_lists, combine_w=combine_w, base=base)


def combine(p, results):
    """Combine per-core device partials into the full [B, OUT] output."""
    caps, slot_exp, tok_lists, combine_w = (
        p["caps"], p["slot_exp"], p["tok_lists"], p["combine_w"])
    out = p["base"].astype(np.float32).copy()
    for c in range(NCORES):
        r = results[c]
        out += r["zt"].astype(np.float32).T
        for j in range(EPC):
            e = slot_exp[c][j]
            toks = tok_lists[e]
            off = sum(caps[:j])
            yre = r["yr"][:, off:off + len(toks)].astype(np.float32)  # [OUT, cnt]
            out[toks] += combine_w[toks, e][:, None] * yre.T
    return out


def kernel(x, task_id, gate_w, w1, b1, w2, b2, w3, b3,
           sw1, sb1, sw2, sb2, sw3, sb3, ow, ob):
    global LAST_RESULTS
    p = prepare(x, task_id, gate_w, w1, b1, w2, b2, w3, b3,
                sw1, sb1, sw2, sb2, sw3, sb3, ow, ob)
    res = run_bass_kernel_spmd(
        p["nc"], p["in_maps"], core_ids=list(range(NCORES)),
        trace=TRACE, **TRACE_KW)
    LAST_RESULTS = res
    return combine(p, res.results)
